# revision 1
# baseline (speedup 1.0000x reference)
"""Trainium2 Bass kernel for BasicAttention — f32r matmuls + PE transposes
+ software-pipelined softmax.

  proj  = keys @ W.T                    (B, NK, DV)
  L     = proj @ values.T               (B, NK, NV)
  A     = softmax(L + mask_bias, -1)
  out   = A @ values                    (B, NK, DV)

Sharding: pure data-parallel over batch — B=16 across 8 cores, 2 batches
per core, no collectives.

Precision: proj and logits matmuls run in float32r (PE reduced-precision
4-byte mode, ~13 effective mantissa bits, 1 cycle/row at moving dim >=
256 — same speed as fp16). Output stage (E, V) is fp16. End-to-end
absmax rel err ~8.5e-3 vs the fp32 reference (dominated by f32r rounding
of the logits operands; the softmax is near-one-hot).

Transposes (W^T, keys^T, values^T) run on the PE itself via
identity-matmul (f32, 2 cycles/row — 128x128 block in ~107 ns), landing
in PSUM and drained by Act copies that produce the f32r SBUF operands
(the BIR verifier requires f32r matmul inputs to be *produced* as f32r).
This removes the fp16 hi/lo split + XBAR transpose + recombine chains
entirely: DVE only does softmax reductions, and the scalar DMA queue
only carries E^T.

Main-loop software pipelining: the output matmuls for k-tile i lag the
logits matmuls by two k-tiles, so each tile's softmax chain (reduce_max
-> Exp -> E^T XBAR transpose) hides under two full L/O rounds of PE
work instead of stalling the PE every tile (which would also drop the
PE out of its max p-state). All Exp banks issue before any E^T
transpose (frees the logits PSUM tile fast), and E^T transposes go on
the SP DMA queue so Act's FIFO never blocks an Exp behind a DMA
dispatch. keys^T prep runs 3 k-blocks ahead and proj 2 ahead inside the
k-tile stream (continuing across batches), giving every PSUM->SBUF
drain a full L/O round to land before its consumer.

Measured: 682-796 us HW across sessions, median ~710 us
(loop-subtraction over a 1001-iteration For_i, dummy-IO variant;
machine-load drift on the shared tunnel dominates the spread), absmax
rel err 8.39e-3; TimelineSim 657 us, PE ~93% busy.
Baseline (3-pass fp16 hi/lo + XBAR transposes, no pipelining):
1910985 ns. Note: f32r-typed PE transposes (1.5 cyc/row on paper)
compile but return garbage on HW — keep these transposes f32.

Per batch: V-tile setup (load -> 8 PE block-transposes -> f32r copies +
masked fp16 vh copy) interleaved with proj k-blocks; then the pipelined
k-tile loop. PSUM: pp/transpose-scratch share 2 banks, L 4, O 2.
"""

import sys
import numpy as np

_TRN_REPO = "/opt/trn_rl_repo"
if _TRN_REPO not in sys.path:
    sys.path.insert(0, _TRN_REPO)

B, NK, NV, DK, DV = 16, 2048, 2048, 1024, 1024
N_CORES = 8
B_LOC = B // N_CORES
P = 128


def build_kernel(b_loc=B_LOC, nk=NK, nv=NV, dk=DK, dv=DV, loop_iters=1,
                 dummy_io=False):
    """Build the single-core Bass program (SPMD-replicated across 8 cores)."""
    import concourse.bacc as bacc
    import concourse.mybir as mybir
    from concourse import tile, masks

    f32 = mybir.dt.float32
    f32r = mybir.dt.float32r
    f16 = mybir.dt.float16
    i32 = mybir.dt.int32
    Exp = mybir.ActivationFunctionType.Exp
    Copy = mybir.ActivationFunctionType.Copy
    X = mybir.AxisListType.X

    KT, NT, DT, VT = nk // P, nv // P, dk // P, dv // P
    KB = min(256, nk)             # proj k-block (moving-dim of proj matmuls)
    KBT = KB // P                 # k-tiles per k-block
    NKB = nk // KB
    LB = min(512, nv)             # logits matmul free-dim block (1 PSUM bank)
    NB = nv // LB
    TPB = LB // P                 # n-tiles per logits bank
    OB = min(512, dv)             # output matmul free-dim block
    OBN = dv // OB

    nc = bacc.Bacc("TRN2", target_bir_lowering=False, debug=False,
                   num_devices=N_CORES)
    if dummy_io:
        # timing-only variant: big tensors live in internal DRAM scratch so
        # nothing heavy ships over the axon tunnel; compute is identical
        keys_d = nc.dram_tensor("keys_s", [b_loc, nk, dk], f32)
        values_d = nc.dram_tensor("values_s", [b_loc, nv, dv], f32)
        w_d = nc.dram_tensor("W_s", [dv, dk], f32)
        out_d = nc.dram_tensor("out_s", [b_loc, nk, dv], f32)
        mask_d = nc.declare_dram_parameter("values_mask", [b_loc, nv], i32,
                                           isOutput=False)
        tok_d = nc.declare_dram_parameter("tok", [1, 1], f32, isOutput=True)
    else:
        keys_d = nc.declare_dram_parameter("keys", [b_loc, nk, dk], f32, isOutput=False)
        values_d = nc.declare_dram_parameter("values", [b_loc, nv, dv], f32, isOutput=False)
        mask_d = nc.declare_dram_parameter("values_mask", [b_loc, nv], i32, isOutput=False)
        w_d = nc.declare_dram_parameter("W", [dv, dk], f32, isOutput=False)
        out_d = nc.declare_dram_parameter("out", [b_loc, nk, dv], f32, isOutput=True)
        tok_d = None

    with tile.TileContext(nc) as tc:
        with (
            tc.tile_pool(name="const", bufs=1) as constp,
            tc.tile_pool(name="wt", bufs=1) as wtp,
            tc.tile_pool(name="vt", bufs=1) as vtp,
            tc.tile_pool(name="stage", bufs=3) as stagep,
            tc.tile_pool(name="ktp", bufs=2) as ktp,
            tc.tile_pool(name="ptp", bufs=3) as ptp,
            tc.tile_pool(name="smp", bufs=1) as smp,
            tc.tile_pool(name="etp", bufs=3) as etp,
            tc.tile_pool(name="otp", bufs=2) as otp,
            tc.tile_pool(name="stats", bufs=6) as stp,
            tc.tile_pool(name="ps_S", bufs=2, space="PSUM") as ps_S,
            tc.tile_pool(name="ps_L", bufs=1, space="PSUM") as ps_L,
            tc.tile_pool(name="ps_O", bufs=1, space="PSUM") as ps_O,
        ):
            ident = constp.tile([P, P], f32, tag="ident")
            masks.make_identity(nc, ident)

            def load_T(dram_ap, dst_pair_view, width, scale_vh=None, vh_dst=None):
                """DRAM [128, width] f32 -> PE block-transposes -> Act f32r
                copies into dst_pair_view(j0) = [P, 2(or 4), 128] SBUF slices.
                Optionally also emit the masked fp16 natural copy (vh)."""
                st = stagep.tile([P, width], f32, tag="stage")
                h = width // 2
                nc.sync.dma_start(st[:, :h], dram_ap[:, :h])
                nc.sync.dma_start(st[:, h:], dram_ap[:, h:])
                nblk = width // P
                for j0 in range(0, nblk, 4):
                    pt = ps_S.tile([P, 4 * P], f32, tag="ps")
                    for j in range(4):
                        nc.tensor.transpose(
                            pt[:, j * P:(j + 1) * P],
                            st[:, (j0 + j) * P:(j0 + j + 1) * P], ident)
                    nc.vector.tensor_copy(dst_pair_view(j0),
                                          pt.rearrange("p (a b) -> p a b", a=4))
                if scale_vh is not None:
                    nc.scalar.activation(vh_dst, st, Copy, scale=scale_vh)

            def body(_i=None):
                # W^T f32r: [P(d), DT, dv] — wt[p, dt, v] = W[v, dt*128+p]
                wt = wtp.tile([P, DT, dv], f32r, tag="wt")
                for vt_i in range(VT):
                    load_T(w_d[vt_i * P:(vt_i + 1) * P, :],
                           lambda j0, vt_i=vt_i: wt[:, j0:j0 + 4,
                                                    vt_i * P:(vt_i + 1) * P],
                           dk)

                def keys_T(b, kb_i):
                    """keys^T f32r for one k-block (PE transposes)."""
                    kT = ktp.tile([P, DT, KB], f32r, tag="kT")
                    for kk in range(KBT):
                        row = (kb_i * KBT + kk) * P
                        load_T(keys_d[b, row:row + P, :],
                               lambda j0, kk=kk: kT[:, j0:j0 + 4,
                                                    kk * P:(kk + 1) * P],
                               dk)
                    return kT

                def proj_mm(kT):
                    """single-pass f32r proj matmuls -> projT."""
                    pT = ptp.tile([P, VT, KB], f32r, tag="pT")
                    for vt_i in range(0, VT, 2):
                        pp = ps_S.tile([P, 2 * KB], f32, tag="ps")
                        for h in range(2):
                            for dt_i in range(DT):
                                nc.tensor.matmul(
                                    pp[:, h * KB:(h + 1) * KB],
                                    lhsT=wt[:, dt_i,
                                            (vt_i + h) * P:(vt_i + h + 1) * P],
                                    rhs=kT[:, dt_i, :],
                                    start=(dt_i == 0), stop=(dt_i == DT - 1))
                        nc.vector.tensor_copy(
                            pT[:, vt_i:vt_i + 2, :],
                            pp.rearrange("p (a b) -> p a b", a=2))
                    return pT

                # global stream of k-blocks across batches: keys^T prep and
                # proj matmuls for batch b+1 run during batch b's k-loop tail
                # so the PE never starves at batch boundaries
                kblocks = [(b, kb) for b in range(b_loc) for kb in range(NKB)]
                next_load = [0]
                kTs, pTs = [], []

                def prep_load():
                    if next_load[0] < len(kblocks):
                        bb, kb = kblocks[next_load[0]]
                        next_load[0] += 1
                        kTs.append(keys_T(bb, kb))

                def prep_proj():
                    if kTs:
                        pTs.append(proj_mm(kTs.pop(0)))

                for b in range(b_loc):
                    # mask as per-partition (n) fp32 scales: [128, NT]
                    mski = stp.tile([P, NT], i32, tag="mski")
                    nc.sync.dma_start(mski, mask_d[b].rearrange("(t p) -> p t", p=P))
                    mskf = stp.tile([P, NT], f32, tag="mskf")
                    nc.scalar.copy(mskf, mski)

                    # V^T f32r (logits rhs) + masked fp16 V (output rhs)
                    vT = vtp.tile([P, VT, nv], f32r, tag="vT")
                    vh = vtp.tile([P, NT, dv], f16, tag="vh")

                    def v_tile(nt_i, vT=vT, vh=vh, mskf=mskf, b=b):
                        load_T(values_d[b, nt_i * P:(nt_i + 1) * P, :],
                               lambda j0, nt_i=nt_i: vT[:, j0:j0 + 4,
                                                        nt_i * P:(nt_i + 1) * P],
                               dv,
                               scale_vh=mskf[:, nt_i:nt_i + 1],
                               vh_dst=vh[:, nt_i, :])

                    if b == 0:
                        # phase B: interleave V-tile setup with keys^T prep +
                        # first proj blocks so the PE always has work
                        prep_load()
                        v_tile(0); v_tile(1)
                        prep_load()
                        v_tile(2); v_tile(3)
                        prep_proj()
                        v_tile(4); v_tile(5)
                        prep_load()
                        v_tile(6); v_tile(7)
                        prep_proj()
                        for nt_i in range(8, NT):
                            v_tile(nt_i)
                    else:
                        # keys^T/proj for this batch's first blocks were
                        # prepped during the previous batch's tail
                        for nt_i in range(NT):
                            v_tile(nt_i)

                    pending = []  # [(eT, r, kt_i)] awaiting output matmuls

                    def emit_output(p, vh=vh, b=b):
                        eT, r, kt_i = p
                        Op = ps_O.tile([P, dv], f32, tag="O")
                        for ob_i in range(OBN):
                            Os = Op[:, ob_i * OB:(ob_i + 1) * OB]
                            for nt_i in range(NT):
                                nc.tensor.matmul(
                                    Os, lhsT=eT[:, nt_i, :],
                                    rhs=vh[:, nt_i, ob_i * OB:(ob_i + 1) * OB],
                                    start=(nt_i == 0), stop=(nt_i == NT - 1))
                        Ot = otp.tile([P, dv], f32, tag="Ot")
                        nc.scalar.activation(Ot, Op, Copy, scale=r)
                        oh = dv // 2
                        nc.sync.dma_start(out_d[b, kt_i * P:(kt_i + 1) * P, :oh],
                                          Ot[:, :oh])
                        nc.sync.dma_start(out_d[b, kt_i * P:(kt_i + 1) * P, oh:],
                                          Ot[:, oh:])

                    for kb_i in range(NKB):
                        pT = pTs.pop(0)

                        for kk in range(KBT):
                            kt_i = kb_i * KBT + kk
                            ks = slice(kk * P, (kk + 1) * P)

                            # L[k, n] single-pass f32r; per-bank max as each
                            # 512-wide PSUM bank's accumulation closes
                            Lp = ps_L.tile([P, nv], f32, tag="L")
                            mx = stp.tile([P, NB], f32, tag="mx")
                            for nb_i in range(NB):
                                Ls = Lp[:, nb_i * LB:(nb_i + 1) * LB]
                                for vt_i in range(VT):
                                    nc.tensor.matmul(
                                        Ls, lhsT=pT[:, vt_i, ks],
                                        rhs=vT[:, vt_i, nb_i * LB:(nb_i + 1) * LB],
                                        start=(vt_i == 0), stop=(vt_i == VT - 1))
                                nc.vector.reduce_max(mx[:, nb_i:nb_i + 1], Ls,
                                                     axis=X)

                            # softmax: E = exp(L - max) fp16, all banks on Act
                            # first (frees Lp for L(kt+1) fast), then the E^T
                            # XBAR transposes on the SP DMA queue
                            negm = stp.tile([P, 1], f32, tag="negm")
                            nc.vector.reduce_max(negm, mx, axis=X, negate=True)
                            Et = smp.tile([P, nv], f16, tag="E")
                            s4 = stp.tile([P, NB], f32, tag="s4")
                            eT = etp.tile([P, NT, P], f16, tag="eT")
                            for nb_i in range(NB):
                                sl = slice(nb_i * LB, (nb_i + 1) * LB)
                                nc.scalar.activation(
                                    Et[:, sl], Lp[:, sl], Exp, bias=negm,
                                    scale=1.0,
                                    accum_out=s4[:, nb_i:nb_i + 1])
                            for nb_i in range(NB):
                                sl = slice(nb_i * LB, (nb_i + 1) * LB)
                                nc.sync.dma_start(
                                    eT[:, nb_i * TPB:(nb_i + 1) * TPB, :],
                                    Et[:, sl], transpose=True)
                            s = stp.tile([P, 1], f32, tag="s")
                            nc.vector.reduce_sum(s, s4, axis=X)
                            r = stp.tile([P, 1], f32, tag="r")
                            nc.vector.reciprocal(r, s)

                            # O(kt-2) issued here: its softmax chain had two
                            # full L/O rounds of PE work to hide under
                            pending.append((eT, r, kt_i))
                            if len(pending) > 2:
                                emit_output(pending.pop(0))

                            if kk == 0:
                                prep_load()
                            elif kk == KBT - 1 and (kb_i + 2 < NKB
                                                    or b + 1 < b_loc):
                                prep_proj()

                    for p in pending:
                        emit_output(p)

            if loop_iters > 1:
                with tc.For_i(0, loop_iters, 1):
                    body()
            else:
                body()
            if tok_d is not None:
                tok = stp.tile([1, 1], f32, tag="tok")
                nc.vector.memset(tok, 0.0)
                nc.sync.dma_start(tok_d[:, :], tok)

    nc.finalize()
    return nc


_NC_CACHE = {}


def _get_nc(**kwargs):
    key = tuple(sorted(kwargs.items()))
    if key not in _NC_CACHE:
        _NC_CACHE[key] = build_kernel(**kwargs)
    return _NC_CACHE[key]


def run(inputs, loop_iters=1, **build_kwargs):
    """Shard full inputs over the 8 cores, run, gather the full output."""
    from concourse.bass_utils import run_bass_kernel_spmd

    nc = _get_nc(loop_iters=loop_iters, **build_kwargs)
    keys = np.ascontiguousarray(inputs["keys"], dtype=np.float32)
    values = np.ascontiguousarray(inputs["values"], dtype=np.float32)
    mask = np.ascontiguousarray(inputs["values_mask"], dtype=np.int32)
    w = np.ascontiguousarray(inputs["W"], dtype=np.float32)

    in_maps = []
    for c in range(N_CORES):
        sl = slice(c * B_LOC, (c + 1) * B_LOC)
        in_maps.append({
            "keys": keys[sl],
            "values": values[sl],
            "values_mask": mask[sl],
            "W": w,
        })
    res = run_bass_kernel_spmd(nc, in_maps, core_ids=list(range(N_CORES)))
    return np.concatenate([res.results[c]["out"] for c in range(N_CORES)], axis=0)


def kernel(**inputs) -> np.ndarray:
    return run(inputs)

